# revision 7
# baseline (speedup 1.0000x reference)
"""DbeCom (clamped-EDT boundary metric) Trainium2 kernel, 8-core SPMD.

Exact reformulation of the jax reference:
  For each image, D = min(sqrt(d2), 10) with d2 the clamped squared EDT.
  Output = thr if sum(pred*(D_gt<thr)) == 0 else
           (sum(pred*D_gt) + sum(gt*D_est)) / (sum(pred) + sum(gt)).

Device algorithm (exact, all in bf16 small integers):
  hd  = run-length distance to nearest fg pixel in the row
        (two tensor_tensor_scan ops, forward/backward)
  c1c = min(hd^2, 100)         # in {0,1,4,...,100}, exact in bf16
  d2  = min_{|di|<=9} c1c[h+di] + di^2   # 19 vertical taps, exact <=100;
        >=100 exactly when the reference clamps to D=10, and the di=0 tap
        guarantees d2 <= 100, so D = sqrt(d2) and [D<10] == [d2<100].
  Vertical taps run on free-dim shifts in a DMA-transposed
  [w mod 128 (partitions), w div 128, h] layout; the +di^2 adds run on the
  scalar engine (activation bias), the mins on the vector engine.
  fg masks are recovered as (c1c == 0).  sum(other*D) = sum(sqrt(other*d2)).
"""

import numpy as np

H_FULL, W = 2048, 4096
NCORES = 8
ROWS = H_FULL // NCORES          # 256 output rows per core
HALO = 9                         # vertical tap radius
BAND = ROWS + 2 * HALO           # 274 input rows per core
SLOTS = 288                      # h-slots (274 used; rounded for xbar)
WC = W // 128                    # 32 column-blocks (w = c*128 + p)
THR = 10.0

_CACHE = {}


def _build_program(reps=1):
    import concourse.bacc as bacc
    import concourse.mybir as mybir
    import concourse.tile as tile
    from contextlib import ExitStack

    dtb = mybir.dt.bfloat16
    dtf = mybir.dt.float32
    A = mybir.AluOpType
    AF = mybir.ActivationFunctionType

    nc = bacc.Bacc("TRN2", target_bir_lowering=False, debug=False,
                   num_devices=NCORES)

    dram_in = {}
    for img in ("g", "p"):
        for t in range(2):
            dram_in[f"{img}{t}"] = nc.dram_tensor(
                f"{img}{t}", [128, W], dtf, kind="ExternalInput").ap()
    dram_in["lo"] = nc.dram_tensor("lo", [64, W], dtf, kind="ExternalInput").ap()
    out_cols = nc.dram_tensor("cols", [128, 8], dtf, kind="ExternalOutput").ap()

    def body(tc, pool):
            nc = tc.nc
            cols = pool.tile([128, 8], dtf, tag="cols", name="cols")
            nc.vector.memset(cols[:], 0.0)

            c1cT = {}
            for img in ("g", "p"):
                c1cT[img] = pool.tile([128, WC, SLOTS], dtb,
                                      tag=f"c1cT{img}", name=f"c1cT{img}")

            # ---------- phase 1: load -> mask -> scans -> hd -> transpose ---
            def phase1_tile(dram, nrows, dsts):
                """dsts: list of (img, src_part_lo, dst_slot_lo, nslots)"""
                raw = pool.tile([nrows, W], dtf, tag="raw", bufs=1, name="raw")
                nc.sync.dma_start(raw[:], dram)
                m = pool.tile([nrows, W], dtb, tag="m", bufs=2, name="m")
                nc.scalar.activation(m[:], raw[:], AF.Copy, scale=-1.0, bias=1.0)
                L = pool.tile([nrows, W], dtb, tag="L", bufs=2, name="L")
                nc.vector.tensor_tensor_scan(
                    L[:], m[:], m[:], 300.0, A.mult, A.add)
                Rt = pool.tile([nrows, W], dtb, tag="R", bufs=2, name="Rt")
                nc.vector.tensor_tensor_scan(
                    Rt[:, ::-1], m[:, ::-1], m[:, ::-1], 300.0, A.mult, A.add)
                hd = pool.tile([nrows, W], dtb, tag="hd", bufs=2, name="hd")
                nc.vector.tensor_tensor(hd[:], L[:], Rt[:], A.min)
                for img, p0, s0, ns in dsts:
                    nc.sync.dma_start_transpose(
                        c1cT[img][:, :, s0:s0 + ns], hd[p0:p0 + ns, :])

            phase1_tile(dram_in["g0"], 128, [("g", 0, 0, 128)])
            phase1_tile(dram_in["g1"], 128, [("g", 0, 128, 128)])
            phase1_tile(dram_in["p0"], 128, [("p", 0, 0, 128)])
            phase1_tile(dram_in["p1"], 128, [("p", 0, 128, 128)])
            phase1_tile(dram_in["lo"], 64,
                        [("g", 0, 256, 32), ("p", 32, 256, 32)])

            # c1c = min(hd^2, 100) in place (square on ACT, clamp on DVE)
            for img in ("g", "p"):
                nc.scalar.activation(c1cT[img][:], c1cT[img][:], AF.Square)
                nc.vector.tensor_scalar(
                    c1cT[img][:], c1cT[img][:], 100.0, None, A.min)

            # ---------- phase 2: vertical 19-tap min-plus -------------------
            def taps(img):
                c = c1cT[img]
                o = HALO

                accD = pool.tile([128, WC, ROWS], dtb, tag="accD", bufs=2,
                                 name="accD")
                ck = pool.tile([128, WC, BAND], dtb, tag="ck", bufs=2, name="ck")
                # ck = c1c + k^2 on ACT (bias add), mins on DVE
                nc.scalar.activation(ck[:], c[:, :, 0:BAND], AF.Copy, bias=1.0)
                nc.vector.tensor_tensor(
                    accD[:], ck[:, :, o - 1: o - 1 + ROWS],
                    ck[:, :, o + 1: o + 1 + ROWS], A.min)
                nc.vector.tensor_tensor(
                    accD[:], accD[:], c[:, :, o: o + ROWS], A.min)
                for k in range(2, HALO + 1):
                    ck = pool.tile([128, WC, BAND], dtb, tag="ck", bufs=2,
                                   name="ck")
                    nc.scalar.activation(ck[:], c[:, :, 0:BAND], AF.Copy,
                                         bias=float(k * k))
                    nc.vector.tensor_tensor(
                        accD[:], accD[:], ck[:, :, o - k: o - k + ROWS], A.min)
                    nc.vector.tensor_tensor(
                        accD[:], accD[:], ck[:, :, o + k: o + k + ROWS], A.min)
                return accD  # d2

            # ---------- phase 3: epilogue ----------------------------------
            col_i = {"Sg": 0, "Sp": 1, "cnt": 2, "fgp": 3, "fgg": 4}

            def epilogue(img, other, d2):
                co = c1cT[other][:, :, HALO: HALO + ROWS]
                scr = pool.tile([128, WC, ROWS], dtb, tag="scr", bufs=1,
                                name="scr")
                # fg_other = (c1c_other == 0); m2 = fg_other * d2 (in place)
                nc.vector.tensor_scalar(scr[:], co, 0.0, None, A.is_equal)
                nc.vector.tensor_tensor(d2[:], scr[:], d2[:], A.mult)
                ci = col_i["Sg" if img == "g" else "Sp"]
                scr2 = pool.tile([128, WC, ROWS], dtb, tag="scr", bufs=1,
                                 name="scr2")
                nc.scalar.activation(
                    scr2[:], d2[:], AF.Sqrt, accum_out=cols[:, ci:ci + 1])
                if img == "g":
                    ci = col_i["cnt"]
                    scr3 = pool.tile([128, WC, ROWS], dtb, tag="scr", bufs=1,
                                     name="scr3")
                    nc.vector.tensor_scalar(
                        scr3[:], d2[:], 99.5, None, A.is_ge, op1=A.add,
                        accum_out=cols[:, ci:ci + 1])
                cs = c1cT[img][:, :, HALO: HALO + ROWS]
                ci = col_i["fgg" if img == "g" else "fgp"]
                scr4 = pool.tile([128, WC, ROWS], dtb, tag="scr", bufs=1,
                                 name="scr4")
                nc.vector.tensor_scalar(
                    scr4[:], cs, 0.0, None, A.is_equal, op1=A.add,
                    accum_out=cols[:, ci:ci + 1])

            d2g = taps("g")
            epilogue("g", "p", d2g)
            d2p = taps("p")
            epilogue("p", "g", d2p)

            nc.sync.dma_start(out_cols, cols[:])

    with tile.TileContext(nc) as tc:
        with ExitStack() as ctx:
            pool = ctx.enter_context(tc.tile_pool(name="pool", bufs=1))
            if reps == 1:
                body(tc, pool)
            else:
                with tc.For_i(0, reps, 1):
                    body(tc, pool)

    nc.compile()
    return nc


def _get_program(reps=1):
    key = ("nc", reps)
    if key not in _CACHE:
        _CACHE[key] = _build_program(reps)
    return _CACHE[key]


def _make_in_maps(gt, pred):
    g = np.ascontiguousarray(gt.reshape(H_FULL, W), dtype=np.float32)
    p = np.ascontiguousarray(pred.reshape(H_FULL, W), dtype=np.float32)
    gp = np.zeros((H_FULL + 2 * HALO, W), np.float32)
    pp = np.zeros((H_FULL + 2 * HALO, W), np.float32)
    gp[HALO:HALO + H_FULL] = g
    pp[HALO:HALO + H_FULL] = p
    in_maps = []
    for c in range(NCORES):
        b = c * ROWS
        lo = np.zeros((64, W), np.float32)
        lo[0:18] = gp[b + 256: b + 274]
        lo[32:50] = pp[b + 256: b + 274]
        in_maps.append({
            "g0": np.ascontiguousarray(gp[b: b + 128]),
            "g1": np.ascontiguousarray(gp[b + 128: b + 256]),
            "p0": np.ascontiguousarray(pp[b: b + 128]),
            "p1": np.ascontiguousarray(pp[b + 128: b + 256]),
            "lo": lo,
        })
    return in_maps


def _combine(results):
    tot = np.zeros(8, np.float64)
    for r in results:
        tot += r["cols"].astype(np.float64).sum(axis=0)
    s_gt, s_est, cnt, fgp, fgg = tot[0], tot[1], tot[2], tot[3], tot[4]
    filt = fgp - cnt
    if filt == 0:
        val = np.float32(THR)
    else:
        val = np.float32(np.float32(s_gt + s_est) / np.float32(fgp + fgg))
    return np.array([val], np.float32)


def _run(gt, pred, reps=1, **kw):
    from concourse.bass_utils import run_bass_kernel_spmd
    nc = _get_program(reps)
    in_maps = _make_in_maps(gt, pred)
    res = run_bass_kernel_spmd(nc, in_maps, list(range(NCORES)), **kw)
    return _combine(res.results), res


def kernel(gt, pred):
    out, _ = _run(gt, pred)
    return out


# revision 9
# speedup vs baseline: 1.0898x; 1.0898x over previous
"""DbeCom (clamped-EDT boundary metric) Trainium2 kernel, 8-core SPMD.

Exact reformulation of the jax reference:
  For each image, D = min(sqrt(d2), 10) with d2 the clamped squared EDT.
  Output = thr if sum(pred*(D_gt<thr)) == 0 else
           (sum(pred*D_gt) + sum(gt*D_est)) / (sum(pred) + sum(gt)).

Device algorithm (exact, all in bf16 small integers):
  hd  = run-length distance to nearest fg pixel in the row
        (two tensor_tensor_scan ops, forward/backward)
  c1c = min(hd^2, 100)         # in {0,1,4,...,100}, exact in bf16
  d2  = min_{|di|<=9} c1c[h+di] + di^2   # 19 vertical taps, exact <=100;
        >=100 exactly when the reference clamps to D=10, and the di=0 tap
        guarantees d2 <= 100, so D = sqrt(d2) and [D<10] == [d2<100].
  Vertical taps run on free-dim shifts in a DMA-transposed
  [w mod 128 (partitions), w div 128, h] layout; the +di^2 adds run on the
  scalar engine (activation bias), the mins on the vector engine.
  fg masks are recovered as (c1c == 0).  sum(other*D) = sum(sqrt(other*d2)).
"""

import numpy as np

H_FULL, W = 2048, 4096
NCORES = 8
ROWS = H_FULL // NCORES          # 256 output rows per core
HALO = 9                         # vertical tap radius
BAND = ROWS + 2 * HALO           # 274 input rows per core
SLOTS = 288                      # h-slots (274 used; rounded for xbar)
WC = W // 128                    # 32 column-blocks (w = c*128 + p)
THR = 10.0

_CACHE = {}


def _build_program(reps=1):
    import concourse.bacc as bacc
    import concourse.mybir as mybir
    import concourse.tile as tile
    from contextlib import ExitStack

    dtb = mybir.dt.bfloat16
    dtf = mybir.dt.float32
    A = mybir.AluOpType
    AF = mybir.ActivationFunctionType

    nc = bacc.Bacc("TRN2", target_bir_lowering=False, debug=False,
                   num_devices=NCORES)

    dram_in = {}
    for img in ("g", "p"):
        for t in range(2):
            dram_in[f"{img}{t}"] = nc.dram_tensor(
                f"{img}{t}", [128, W], dtf, kind="ExternalInput").ap()
    dram_in["lo"] = nc.dram_tensor("lo", [64, W], dtf, kind="ExternalInput").ap()
    out_cols = nc.dram_tensor("cols", [128, 8], dtf, kind="ExternalOutput").ap()

    def body(tc, pool):
            nc = tc.nc
            cols = pool.tile([128, 8], dtf, tag="cols", name="cols")
            nc.vector.memset(cols[:], 0.0)

            c1cT = {}
            for img in ("g", "p"):
                c1cT[img] = pool.tile([128, WC, SLOTS], dtb,
                                      tag=f"c1cT{img}", name=f"c1cT{img}")

            # ---------- phase 1: load -> mask -> scans -> hd -> transpose ---
            def phase1_tile(dram, nrows, dsts):
                """dsts: list of (img, src_part_lo, dst_slot_lo, nslots)"""
                raw = pool.tile([nrows, W], dtf, tag="raw", bufs=2, name="raw")
                nc.sync.dma_start(raw[:], dram)
                m = pool.tile([nrows, W], dtb, tag="m", bufs=1, name="m")
                nc.scalar.activation(m[:], raw[:], AF.Copy, scale=-1.0, bias=1.0)
                L = pool.tile([nrows, W], dtb, tag="L", bufs=1, name="L")
                nc.vector.tensor_tensor_scan(
                    L[:], m[:], m[:], 300.0, A.mult, A.add)
                Rt = pool.tile([nrows, W], dtb, tag="R", bufs=1, name="Rt")
                nc.vector.tensor_tensor_scan(
                    Rt[:, ::-1], m[:, ::-1], m[:, ::-1], 300.0, A.mult, A.add)
                hd = pool.tile([nrows, W], dtb, tag="hd", bufs=2, name="hd")
                nc.vector.tensor_tensor(hd[:], L[:], Rt[:], A.min)
                for img, p0, s0, ns in dsts:
                    nc.sync.dma_start_transpose(
                        c1cT[img][:, :, s0:s0 + ns], hd[p0:p0 + ns, :])

            phase1_tile(dram_in["g0"], 128, [("g", 0, 0, 128)])
            phase1_tile(dram_in["g1"], 128, [("g", 0, 128, 128)])
            phase1_tile(dram_in["p0"], 128, [("p", 0, 0, 128)])
            phase1_tile(dram_in["p1"], 128, [("p", 0, 128, 128)])
            phase1_tile(dram_in["lo"], 64,
                        [("g", 0, 256, 32), ("p", 32, 256, 32)])

            # c1c = min(hd^2, 100) in place (square on ACT, clamp on DVE)
            for img in ("g", "p"):
                nc.scalar.activation(c1cT[img][:], c1cT[img][:], AF.Square)
                nc.vector.tensor_scalar(
                    c1cT[img][:], c1cT[img][:], 100.0, None, A.min)

            # ---------- phase 2: vertical 19-tap min-plus -------------------
            # 9 independent pair-mins (+-k via one shared ck = c1c + k^2)
            # then a binary-tree reduction; both images interleaved for ILP.
            def taps_both():
                o = HALO
                vals = {"g": [], "p": []}   # 10 values per image to tree-min
                for img in ("g", "p"):
                    vals[img].append(c1cT[img][:, :, o: o + ROWS])
                for k in range(1, HALO + 1):
                    for img in ("g", "p"):
                        c = c1cT[img]
                        ck = pool.tile([128, WC, BAND], dtb, tag="ck", bufs=4,
                                       name="ck")
                        nc.scalar.activation(ck[:], c[:, :, 0:BAND], AF.Copy,
                                             bias=float(k * k))
                        pk = pool.tile([128, WC, ROWS], dtb, tag=f"pk{img}",
                                       bufs=5, name="pk")
                        nc.vector.tensor_tensor(
                            pk[:], ck[:, :, o - k: o - k + ROWS],
                            ck[:, :, o + k: o + k + ROWS], A.min)
                        vals[img].append(pk[:])
                # tree reduce 10 -> 1 per image, interleaved
                while len(vals["g"]) > 1:
                    for img in ("g", "p"):
                        v = vals[img]
                        nv = []
                        for i in range(0, len(v) - 1, 2):
                            if len(v) == 2:
                                t = pool.tile([128, WC, ROWS], dtb,
                                              tag=f"accD{img}", bufs=1,
                                              name="acc")
                            else:
                                t = pool.tile([128, WC, ROWS], dtb,
                                              tag=f"pk{img}", bufs=5,
                                              name="tr")
                            nc.vector.tensor_tensor(t[:], v[i], v[i + 1], A.min)
                            nv.append(t[:])
                        if len(v) % 2:
                            nv.append(v[-1])
                        vals[img] = nv
                return vals["g"][0], vals["p"][0]

            # ---------- phase 3: epilogue ----------------------------------
            col_i = {"Sg": 0, "Sp": 1, "cnt": 2, "fgp": 3, "fgg": 4}

            def epilogue(img, other, d2):
                co = c1cT[other][:, :, HALO: HALO + ROWS]
                scr = pool.tile([128, WC, ROWS], dtb, tag="scr", bufs=1,
                                name="scr")
                # fg_other = (c1c_other == 0); m2 = fg_other * d2 (in place)
                nc.vector.tensor_scalar(scr[:], co, 0.0, None, A.is_equal)
                nc.gpsimd.tensor_tensor(d2[:], scr[:], d2[:], A.mult)
                ci = col_i["Sg" if img == "g" else "Sp"]
                scr2 = pool.tile([128, WC, ROWS], dtb, tag="scr", bufs=1,
                                 name="scr2")
                nc.scalar.activation(
                    scr2[:], d2[:], AF.Sqrt, accum_out=cols[:, ci:ci + 1])
                if img == "g":
                    ci = col_i["cnt"]
                    scr3 = pool.tile([128, WC, ROWS], dtb, tag="scr", bufs=1,
                                     name="scr3")
                    nc.vector.tensor_scalar(
                        scr3[:], d2[:], 99.5, None, A.is_ge, op1=A.add,
                        accum_out=cols[:, ci:ci + 1])
                cs = c1cT[img][:, :, HALO: HALO + ROWS]
                ci = col_i["fgg" if img == "g" else "fgp"]
                scr4 = pool.tile([128, WC, ROWS], dtb, tag="scr", bufs=1,
                                 name="scr4")
                nc.vector.tensor_scalar(
                    scr4[:], cs, 0.0, None, A.is_equal, op1=A.add,
                    accum_out=cols[:, ci:ci + 1])

            d2g, d2p = taps_both()
            epilogue("g", "p", d2g)
            epilogue("p", "g", d2p)

            nc.sync.dma_start(out_cols, cols[:])

    with tile.TileContext(nc) as tc:
        with ExitStack() as ctx:
            pool = ctx.enter_context(tc.tile_pool(name="pool", bufs=1))
            if reps == 1:
                body(tc, pool)
            else:
                with tc.For_i(0, reps, 1):
                    body(tc, pool)

    nc.compile()
    return nc


def _get_program(reps=1):
    key = ("nc", reps)
    if key not in _CACHE:
        _CACHE[key] = _build_program(reps)
    return _CACHE[key]


def _make_in_maps(gt, pred):
    g = np.ascontiguousarray(gt.reshape(H_FULL, W), dtype=np.float32)
    p = np.ascontiguousarray(pred.reshape(H_FULL, W), dtype=np.float32)
    gp = np.zeros((H_FULL + 2 * HALO, W), np.float32)
    pp = np.zeros((H_FULL + 2 * HALO, W), np.float32)
    gp[HALO:HALO + H_FULL] = g
    pp[HALO:HALO + H_FULL] = p
    in_maps = []
    for c in range(NCORES):
        b = c * ROWS
        lo = np.zeros((64, W), np.float32)
        lo[0:18] = gp[b + 256: b + 274]
        lo[32:50] = pp[b + 256: b + 274]
        in_maps.append({
            "g0": np.ascontiguousarray(gp[b: b + 128]),
            "g1": np.ascontiguousarray(gp[b + 128: b + 256]),
            "p0": np.ascontiguousarray(pp[b: b + 128]),
            "p1": np.ascontiguousarray(pp[b + 128: b + 256]),
            "lo": lo,
        })
    return in_maps


def _combine(results):
    tot = np.zeros(8, np.float64)
    for r in results:
        tot += r["cols"].astype(np.float64).sum(axis=0)
    s_gt, s_est, cnt, fgp, fgg = tot[0], tot[1], tot[2], tot[3], tot[4]
    filt = fgp - cnt
    if filt == 0:
        val = np.float32(THR)
    else:
        val = np.float32(np.float32(s_gt + s_est) / np.float32(fgp + fgg))
    return np.array([val], np.float32)


def _run(gt, pred, reps=1, **kw):
    from concourse.bass_utils import run_bass_kernel_spmd
    nc = _get_program(reps)
    in_maps = _make_in_maps(gt, pred)
    res = run_bass_kernel_spmd(nc, in_maps, list(range(NCORES)), **kw)
    return _combine(res.results), res


def kernel(gt, pred):
    out, _ = _run(gt, pred)
    return out
